# revision 31
# baseline (speedup 1.0000x reference)
# GQA attention layer (B=1, S=2048, HID=2560, H=32, HKV=8, D=128) on 8 TRN2
# NeuronCores. Tensor-parallel over kv-head groups: core c owns kv head c and
# its 4 query heads (Wq/Wk/Wv row shards, Wo column shard). The o_proj
# partials are combined with an on-device ReduceScatter over the sequence
# axis (4 chunks, overlapped with compute); the host reassembles the
# sequence-sharded outputs.
#
# Per-core dataflow (all matmuls bf16 -> fp32 PSUM):
#   1. QKV projection from X^T tiles (s-major output layout), per-head
#      RMSNorm + RoPE on DVE, PE-transpose of Q/K into [d, s] layout.
#      rsqrt(var) is computed as exp(-0.5*ln(var)) on ACT so every ACT
#      function used by the kernel ({Ln, Exp, Copy}) lives in one act table
#      (no table reloads).
#   2. Scores are computed transposed (S^T[k, q] = K Q^T) so that the
#      P^T @ V matmul needs no transpose of the 16.8M-element prob matrix.
#      Scores land in 2-bank PSUM tiles (two 512-col matmuls) so one ACT
#      exp instruction covers 1024 columns. Softmax denominators: a DVE
#      binary tree compresses exp tiles 16 -> 1 along kt, then a single
#      all-ones [128,128] stationary matmul broadcasts the column sums to
#      every partition (vs 16 ones-matmuls = 67us of PE in the baseline).
#      reciprocal_approx_fast (~18 bits) inverts the sums.
#   3. o_proj per 512-row chunk (evictions all on DVE), ReduceScatter per
#      chunk in bf16 overlapped with the next chunk's compute. The
#      rs_out -> out DMAs are emitted at the END of the program so a
#      collective-waiting DMA never parks at the head of the sync queue and
#      head-of-line-blocks the next chunk's eviction DMAs (a 44us PE stall
#      in the baseline).
import sys

if "/opt/trn_rl_repo" not in sys.path:
    sys.path.insert(0, "/opt/trn_rl_repo")

import numpy as np
import ml_dtypes

import concourse.bacc as bacc
import concourse.mybir as mybir
import concourse.tile as tile
from concourse import bass_utils, masks

BF16 = mybir.dt.bfloat16
F32 = mybir.dt.float32

B, S, HID = 1, 2048, 2560
H, HKV, D = 32, 8, 128
G = H // HKV  # q heads per kv head (= per core)
NC = 8  # cores
DQ = G * D  # per-core q width (512)
EPS = 1e-6
SCALE = 1.0 / float(np.sqrt(D))

ST = 128          # s positions per compute tile
N_ST = S // ST    # 16
HC = HID // 128   # 20 contraction chunks
XL = 256          # s positions per X^T DMA load tile
N_XL = S // XL    # 8
QC = 512          # q positions per attention unit
N_QC = S // QC    # 4 (also the ReduceScatter chunk count)
N_KT = S // 128   # 16 k tiles per attention unit
N_KP = N_KT // 2  # 8 kt pairs (exp granularity)
NO = HID // 512   # 5 o_proj free-dim chunks

_NC_CACHE = None


def _build(reps: int = 1, single: bool = False):
    nc = bacc.Bacc(
        "TRN2", target_bir_lowering=False, debug=False,
        num_devices=(1 if single else NC),
    )

    # partition-major X^T tiles: each partition's line is HC*XL*2 = 10 KB
    # contiguous, so the load DMAs run at full descriptor efficiency
    xt_d = nc.dram_tensor("xt", [N_XL, 128, HC, XL], BF16, kind="ExternalInput").ap()
    wq_d = nc.dram_tensor("wq", [HC, 128, DQ], BF16, kind="ExternalInput").ap()
    wkv_d = nc.dram_tensor("wkv", [HC, 128, 2 * D], BF16, kind="ExternalInput").ap()
    wo_d = nc.dram_tensor("wo", [128, G, HID], BF16, kind="ExternalInput").ap()
    cwq_d = nc.dram_tensor("cwq", [N_ST, 128, D], F32, kind="ExternalInput").ap()
    swq_d = nc.dram_tensor("swq", [N_ST, 128, D], F32, kind="ExternalInput").ap()
    cwk_d = nc.dram_tensor("cwk", [N_ST, 128, D], F32, kind="ExternalInput").ap()
    swk_d = nc.dram_tensor("swk", [N_ST, 128, D], F32, kind="ExternalInput").ap()
    out_d = nc.dram_tensor("out", [S // NC, HID], BF16, kind="ExternalOutput").ap()

    with tile.TileContext(nc) as tc:
        with (
            tc.tile_pool(name="const", bufs=1) as cpool,
            tc.tile_pool(name="xt", bufs=2) as xt_pool,
            tc.tile_pool(name="cs", bufs=8) as cs_pool,
            tc.tile_pool(name="qw", bufs=5) as qw_pool,
            tc.tile_pool(name="kw", bufs=6) as kw_pool,
            tc.tile_pool(name="ro", bufs=2) as ro_pool,
            tc.tile_pool(name="sm", bufs=4) as sm_pool,
            tc.tile_pool(name="ep", bufs=2) as ep_pool,
            tc.tile_pool(name="tr", bufs=2) as tr_pool,
            tc.tile_pool(name="ot", bufs=8) as ot_pool,
            tc.tile_pool(name="ob", bufs=4) as ob_pool,
            tc.tile_pool(name="psA", bufs=2, space="PSUM") as psA,
            tc.tile_pool(name="psB", bufs=2, space="PSUM") as psB,
            tc.tile_pool(name="psC", bufs=2, space="PSUM") as psC,
            tc.tile_pool(name="dram", bufs=1, space="DRAM") as dram,
        ):
            for _rep in range(reps):
                # Pin the ACT table to natural_log_exp_and_others (id 6 in
                # act_info.json): it contains every ACT function this kernel
                # uses ({Ln, Exp, Copy}), so the compiler's table-load pass
                # finds the table already resident on every path and inserts
                # no per-iteration reloads (the baseline paid ~27 1.3us
                # reloads thrashing between the sqrt and exp tables).
                nc.scalar.add_instruction(
                    mybir.InstLoadActFuncSet(
                        name=nc.get_next_instruction_name(),
                        ins=[],
                        outs=[],
                        act_func_set_id=6,
                    )
                )
                # ---- resident constants / weights ----
                ident = cpool.tile([128, 128], BF16, tag="ident")
                masks.make_identity(nc, ident[:])
                # all-ones stationary: one matmul on the kt-compressed exp
                # sums yields the softmax denominator replicated across all
                # 128 partitions (free bcast)
                ones_k = cpool.tile([128, 128], BF16, tag="ones_k")
                nc.vector.memset(ones_k[:], 1.0)

                # first X^T tile before the weight block so the PE can start
                # as soon as xt[0] + wq[0] land; split in half so the first
                # ST only waits on 0.65 MB
                xt_t = xt_pool.tile([128, HC, XL], BF16, tag="xt")
                nc.sync.dma_start(xt_t[:], xt_d[0])

                # per-chunk weight tiles so the first matmul only waits on
                # chunk 0, not the whole 6.5 MB weight load
                wq_t = []
                wkv_t = []
                xt_next = None
                for ch in range(HC):
                    # split the 6.5 MB weight load across two DMA queues
                    # (scalar engine is idle at start and these DMAs have no
                    # wait conditions, so no head-of-line risk): the st0/st1
                    # matmuls were starving ~5us waiting for chunk arrival
                    weng = nc.sync if ch % 2 == 0 else nc.scalar
                    w1 = cpool.tile([128, DQ], BF16, tag=f"wq{ch}")
                    weng.dma_start(w1[:], wq_d[ch])
                    wq_t.append(w1)
                    w2 = cpool.tile([128, 2 * D], BF16, tag=f"wkv{ch}")
                    weng.dma_start(w2[:], wkv_d[ch])
                    wkv_t.append(w2)
                    if ch == 5:
                        xt_next = xt_pool.tile([128, HC, XL], BF16, tag="xt")
                        nc.sync.dma_start(xt_next[:], xt_d[1])

                qt_sb = cpool.tile([128, G, S], BF16, tag="qt")   # Q^T  [d, h, s]
                kt_sb = cpool.tile([128, S], BF16, tag="kt")      # K^T  [d, s]
                v_sb = cpool.tile([128, N_KT, D], BF16, tag="v")  # V    [s%128, kt, d]

                # ================= phase 1: QKV + norm + rope + transpose ======
                # first attention unit's ep tile; its scores/exp are emitted
                # inside the phase-1 loop as soon as each kt PAIR's K^T lands
                ep00 = ep_pool.tile([128, N_KT, QC], BF16, tag="ep")
                t8_00 = tr_pool.tile([128, N_KP, QC], BF16, tag="t8")
                for st in range(N_ST):
                    if st % (XL // ST) == 0 and st > 0:
                        if st // (XL // ST) == 1:
                            xt_t = xt_next
                        else:
                            xt_t = xt_pool.tile([128, HC, XL], BF16, tag="xt")
                            nc.sync.dma_start(xt_t[:], xt_d[st // (XL // ST)])
                    soff = (st % (XL // ST)) * ST

                    # cos/sin loads on the gpsimd queue (idle until phase 2):
                    # on the sync queue they drained behind the 6.5 MB weight
                    # block and contended with the X^T tile loads
                    cwq_t = cs_pool.tile([128, D], F32, tag="cs")
                    nc.gpsimd.dma_start(cwq_t[:], cwq_d[st])
                    swq_t = cs_pool.tile([128, D], F32, tag="cs")
                    nc.gpsimd.dma_start(swq_t[:], swq_d[st])
                    cwk_t = cs_pool.tile([128, D], F32, tag="cs")
                    nc.gpsimd.dma_start(cwk_t[:], cwk_d[st])
                    swk_t = cs_pool.tile([128, D], F32, tag="cs")
                    nc.gpsimd.dma_start(swk_t[:], swk_d[st])

                    q_ps = psA.tile([128, DQ], F32, tag="s2")
                    kv_ps = psB.tile([128, 2 * D], F32, tag="b")
                    for ch in range(HC):
                        lhs = xt_t[:, ch, soff : soff + ST]
                        nc.tensor.matmul(
                            q_ps[:], lhs, wq_t[ch][:],
                            start=(ch == 0), stop=(ch == HC - 1),
                        )
                        nc.tensor.matmul(
                            kv_ps[:], lhs, wkv_t[ch][:],
                            start=(ch == 0), stop=(ch == HC - 1),
                        )

                    # evictions (scalar engine)
                    q_sb = qw_pool.tile([128, DQ], F32, tag="qw")
                    nc.scalar.copy(q_sb[:], q_ps[:])
                    k_sb = kw_pool.tile([128, D], F32, tag="kw")
                    nc.scalar.copy(k_sb[:], kv_ps[:, 0:D])
                    nc.scalar.copy(v_sb[:, st, :], kv_ps[:, D : 2 * D])

                    # ---- RMSNorm (per head) ----
                    sq = qw_pool.tile([128, DQ], F32, tag="qw")
                    nc.vector.tensor_mul(sq[:], q_sb[:], q_sb[:])
                    ssq = sm_pool.tile([128, G + 1], F32, tag="sm")
                    nc.vector.tensor_reduce(
                        ssq[:, 0:G], sq[:].rearrange("p (h d) -> p h d", d=D),
                        axis=mybir.AxisListType.X, op=mybir.AluOpType.add,
                    )
                    ksq = kw_pool.tile([128, D], F32, tag="kw")
                    nc.vector.tensor_mul(ksq[:], k_sb[:], k_sb[:])
                    nc.vector.tensor_reduce(
                        ssq[:, G : G + 1], ksq[:].unsqueeze(1),
                        axis=mybir.AxisListType.X, op=mybir.AluOpType.add,
                    )
                    var = sm_pool.tile([128, G + 1], F32, tag="sm")
                    nc.vector.tensor_scalar(
                        var[:], ssq[:], 1.0 / D, EPS,
                        op0=mybir.AluOpType.mult, op1=mybir.AluOpType.add,
                    )
                    # rsqrt(var) = exp(-0.5 * ln(var)): keeps ACT on the
                    # {Ln, Exp, Copy} table set (no act-table reloads)
                    lnv = sm_pool.tile([128, G + 1], F32, tag="sm")
                    nc.scalar.activation(
                        lnv[:], var[:], mybir.ActivationFunctionType.Ln
                    )
                    rq = sm_pool.tile([128, G + 1], F32, tag="sm")
                    nc.scalar.activation(
                        rq[:], lnv[:], mybir.ActivationFunctionType.Exp,
                        scale=-0.5,
                    )
                    rk = rq

                    # ---- normalize + rope (DVE) ----
                    qn = qw_pool.tile([128, DQ], F32, tag="qw")
                    qn3 = qn[:].rearrange("p (h d) -> p h d", d=D)
                    nc.vector.tensor_tensor(
                        qn3, q_sb[:].rearrange("p (h d) -> p h d", d=D),
                        rq[:, 0:G].unsqueeze(2).to_broadcast([128, G, D]),
                        op=mybir.AluOpType.mult,
                    )
                    t1 = qw_pool.tile([128, DQ], F32, tag="qw")
                    t13 = t1[:].rearrange("p (h d) -> p h d", d=D)
                    cwq3 = cwq_t[:].unsqueeze(1).to_broadcast([128, G, D])
                    swq3 = swq_t[:].unsqueeze(1).to_broadcast([128, G, D])
                    nc.vector.tensor_tensor(t13, qn3, cwq3, op=mybir.AluOpType.mult)
                    u = qw_pool.tile([128, DQ], F32, tag="qw")
                    u3 = u[:].rearrange("p (h d) -> p h d", d=D)
                    hd = D // 2
                    nc.vector.tensor_tensor(
                        u3[:, :, 0:hd], qn3[:, :, hd:D], swq3[:, :, 0:hd],
                        op=mybir.AluOpType.mult,
                    )
                    nc.vector.tensor_tensor(
                        u3[:, :, hd:D], qn3[:, :, 0:hd], swq3[:, :, hd:D],
                        op=mybir.AluOpType.mult,
                    )
                    qro = ro_pool.tile([128, DQ], BF16, tag="qro")
                    qro3 = qro[:].rearrange("p (h d) -> p h d", d=D)
                    nc.vector.tensor_sub(qro3[:, :, 0:hd], t13[:, :, 0:hd], u3[:, :, 0:hd])
                    nc.vector.tensor_add(qro3[:, :, hd:D], t13[:, :, hd:D], u3[:, :, hd:D])

                    kn = kw_pool.tile([128, D], F32, tag="kw")
                    nc.vector.tensor_tensor(
                        kn[:], k_sb[:],
                        rk[:, G : G + 1].to_broadcast([128, D]),
                        op=mybir.AluOpType.mult,
                    )
                    kt1 = kw_pool.tile([128, D], F32, tag="kw")
                    nc.vector.tensor_tensor(kt1[:], kn[:], cwk_t[:], op=mybir.AluOpType.mult)
                    ku = kw_pool.tile([128, D], F32, tag="kw")
                    nc.vector.tensor_tensor(
                        ku[:, 0:hd], kn[:, hd:D], swk_t[:, 0:hd], op=mybir.AluOpType.mult
                    )
                    nc.vector.tensor_tensor(
                        ku[:, hd:D], kn[:, 0:hd], swk_t[:, hd:D], op=mybir.AluOpType.mult
                    )
                    kro = ro_pool.tile([128, D], BF16, tag="kro")
                    nc.vector.tensor_sub(kro[:, 0:hd], kt1[:, 0:hd], ku[:, 0:hd])
                    nc.vector.tensor_add(kro[:, hd:D], kt1[:, hd:D], ku[:, hd:D])

                    # ---- transpose Q heads + K into [d, s] ----
                    for h in range(G):
                        tp = psC.tile([128, 128], BF16, tag="c")
                        nc.tensor.transpose(tp[:], qro[:, h * D : (h + 1) * D], ident[:])
                        nc.scalar.copy(qt_sb[:, h, st * ST : (st + 1) * ST], tp[:])
                    tp = psC.tile([128, 128], BF16, tag="c")
                    nc.tensor.transpose(tp[:], kro[:], ident[:])
                    nc.scalar.copy(kt_sb[:, st * ST : (st + 1) * ST], tp[:])

                    # interleave unit (0,0)'s scores+exp per completed kt PAIR
                    if st >= 5 and st % 2 == 1:
                        kp = (st - 5) // 2
                        s_ps = psA.tile([128, 2 * QC], F32, tag="s2")
                        for j in range(2):
                            kt = 2 * kp + j
                            nc.tensor.matmul(
                                s_ps[:, j * QC : (j + 1) * QC],
                                kt_sb[:, kt * 128 : (kt + 1) * 128],
                                qt_sb[:, 0, 0:QC],
                                start=True, stop=True,
                            )
                        nc.scalar.activation(
                            ep00[:].rearrange("p k q -> p (k q)")[
                                :, 2 * kp * QC : (2 * kp + 2) * QC
                            ],
                            s_ps[:],
                            mybir.ActivationFunctionType.Exp, scale=SCALE,
                        )
                        with nc.allow_low_precision("bf16 softmax partials"):
                            nc.vector.tensor_add(
                                t8_00[:, kp, :],
                                ep00[:, 2 * kp, :], ep00[:, 2 * kp + 1, :],
                            )
                            if kp == 3:
                                nc.vector.tensor_add(
                                    t8_00[:, 0:2, :],
                                    t8_00[:, 0:2, :], t8_00[:, 2:4, :],
                                )
                            if kp == 5:
                                nc.vector.tensor_add(
                                    t8_00[:, 0, :], t8_00[:, 0, :], t8_00[:, 1, :]
                                )
                                nc.vector.tensor_add(
                                    t8_00[:, 4, :], t8_00[:, 4, :], t8_00[:, 5, :]
                                )

                # ================= phase 2: attention + o_proj + RS ============
                # wo is first needed ~10us into phase 2; load it behind the
                # phase-1 traffic instead of ahead of it
                wo_sb = cpool.tile([128, G, HID], BF16, tag="wo")
                nc.sync.dma_start(wo_sb[:], wo_d)
                for qc in range(N_QC):
                    ot_tiles = []
                    for h in range(G):
                        if qc == 0 and h == 0:
                            ep = ep00
                            t8 = t8_00
                            kp_start = N_KP - 2
                        else:
                            ep = ep_pool.tile([128, N_KT, QC], BF16, tag="ep")
                            t8 = tr_pool.tile([128, N_KP, QC], BF16, tag="t8")
                            kp_start = 0
                        epf = ep[:].rearrange("p k q -> p (k q)")
                        for kp in range(kp_start, N_KP):
                            s_ps = psA.tile([128, 2 * QC], F32, tag="s2")
                            for j in range(2):
                                kt = 2 * kp + j
                                nc.tensor.matmul(
                                    s_ps[:, j * QC : (j + 1) * QC],
                                    kt_sb[:, kt * 128 : (kt + 1) * 128],
                                    qt_sb[:, h, qc * QC : (qc + 1) * QC],
                                    start=True, stop=True,
                                )
                            nc.scalar.activation(
                                epf[:, 2 * kp * QC : (2 * kp + 2) * QC],
                                s_ps[:],
                                mybir.ActivationFunctionType.Exp, scale=SCALE,
                            )
                            # incremental pair-sum right after each exp, with
                            # progressive folds at kp=3/5: after the LAST exp
                            # only 3 small adds remain before the sums matmul,
                            # so the denominator chain no longer head-blocks
                            # the PE queue ahead of each chunk's o_proj
                            with nc.allow_low_precision("bf16 softmax partials"):
                                nc.vector.tensor_add(
                                    t8[:, kp, :],
                                    ep[:, 2 * kp, :], ep[:, 2 * kp + 1, :],
                                )
                                if kp == 3:
                                    # pairs 0-3 -> 2 slices
                                    nc.vector.tensor_add(
                                        t8[:, 0:2, :], t8[:, 0:2, :], t8[:, 2:4, :]
                                    )
                                if kp == 5:
                                    # pairs 0-3 -> 1 slice; pairs 4,5 -> 1
                                    nc.vector.tensor_add(
                                        t8[:, 0, :], t8[:, 0, :], t8[:, 1, :]
                                    )
                                    nc.vector.tensor_add(
                                        t8[:, 4, :], t8[:, 4, :], t8[:, 5, :]
                                    )
                        pv_ps = psB.tile([128, QC], F32, tag="b")
                        for kt in range(N_KT):
                            nc.tensor.matmul(
                                pv_ps[:], v_sb[:, kt, :], ep[:, kt, :],
                                start=(kt == 0), stop=(kt == N_KT - 1),
                            )
                        # finish the denominator: slices 0 (pairs 0-3),
                        # 4 (pairs 4,5), 6, 7 remain after the in-loop folds
                        with nc.allow_low_precision(
                            "bf16 partial sums of positive exp values; "
                            "relative error ~0.3% on the softmax denominator"
                        ):
                            nc.vector.tensor_add(
                                t8[:, 6, :], t8[:, 6, :], t8[:, 7, :]
                            )
                            nc.vector.tensor_add(
                                t8[:, 4, :], t8[:, 4, :], t8[:, 6, :]
                            )
                            nc.vector.tensor_add(
                                t8[:, 0, :], t8[:, 0, :], t8[:, 4, :]
                            )
                        sums_ps = psC.tile([128, QC], F32, tag="c")
                        nc.tensor.matmul(
                            sums_ps[:], ones_k[:], t8[:, 0, :],
                            start=True, stop=True,
                        )
                        # sums_ps rows are all identical (ones stationary) —
                        # approx reciprocal (~18 bits) is plenty for softmax
                        rb = sm_pool.tile([128, QC], F32, tag="rb", bufs=2)
                        nc.vector.reciprocal_approx_fast(rb[:], sums_ps[:])
                        ot = ot_pool.tile([128, QC], BF16, tag="ot")
                        nc.vector.tensor_tensor(
                            ot[:], pv_ps[:], rb[:], op=mybir.AluOpType.mult
                        )
                        ot_tiles.append(ot)

                    # o_proj for this 512-row chunk; one full-chunk
                    # ReduceScatter (2.6 MB/rank -> RDH regime, better bus rate)
                    RROWS = QC // NC  # 64 output rows per core per RS
                    rs_in = dram.tile([QC, HID], BF16, tag=f"rsin{qc}")
                    rs_out = dram.tile([RROWS, HID], BF16, tag=f"rsout{qc}")
                    for si in range(QC // ST):
                        sst = si
                        ob = ob_pool.tile([128, HID], BF16, tag="ob")
                        for no in range(NO):
                            y_ps = psB.tile([128, 512], F32, tag="b")
                            for h in range(G):
                                nc.tensor.matmul(
                                    y_ps[:],
                                    ot_tiles[h][:, sst * ST : (sst + 1) * ST],
                                    wo_sb[:, h, no * 512 : (no + 1) * 512],
                                    start=(h == 0), stop=(h == G - 1),
                                )
                            nc.vector.tensor_copy(
                                ob[:, no * 512 : (no + 1) * 512], y_ps[:]
                            )
                            nc.sync.dma_start(
                                rs_in[si * ST : (si + 1) * ST,
                                      no * 512 : (no + 1) * 512],
                                ob[:, no * 512 : (no + 1) * 512],
                            )

                    orow = qc * (QC // NC)
                    if single:
                        nc.sync.dma_start(
                            out_d[orow : orow + RROWS, :], rs_in[0:RROWS, :]
                        )
                    else:
                        nc.gpsimd.collective_compute(
                            "ReduceScatter",
                            mybir.AluOpType.add,
                            replica_groups=[list(range(NC))],
                            ins=[rs_in.opt()],
                            outs=[rs_out.opt()],
                        )
                        # rs_out -> out DMA on the GPSIMD queue: it waits for
                        # the collective, and the only things behind it there
                        # are later collectives (which the in-order CC stream
                        # serializes anyway). On the sync queue this DMA
                        # head-of-line-blocked the next chunk's o_proj
                        # eviction DMAs -> 44us PE stall in the baseline.
                        nc.gpsimd.dma_start(
                            out_d[orow : orow + RROWS, :], rs_out[:]
                        )

    nc.compile()
    return nc


def _get_nc():
    global _NC_CACHE
    if _NC_CACHE is None:
        _NC_CACHE = _build()
    return _NC_CACHE


def make_in_maps(inputs):
    X = np.asarray(inputs["hidden_states"], dtype=np.float32).reshape(S, HID)
    freqs = np.asarray(inputs["freqs_cis"], dtype=np.float32)
    Wq = np.asarray(inputs["Wq"], dtype=np.float32)
    Wk = np.asarray(inputs["Wk"], dtype=np.float32)
    Wv = np.asarray(inputs["Wv"], dtype=np.float32)
    Wo = np.asarray(inputs["Wo"], dtype=np.float32)
    qw = np.asarray(inputs["q_norm_w"], dtype=np.float32)
    kw = np.asarray(inputs["k_norm_w"], dtype=np.float32)

    bf = ml_dtypes.bfloat16
    # X^T load tiles, partition-major: (L, p, ch, s) = X[L*XL+s, ch*128+p]
    xt = np.ascontiguousarray(
        X.reshape(N_XL, XL, HC, 128).transpose(0, 3, 2, 1).astype(bf)
    )
    cos, sin = freqs[0], freqs[1]  # [S, D]
    cwq = np.ascontiguousarray((cos * qw[None, :]).reshape(N_ST, 128, D))
    swq = np.ascontiguousarray((sin * np.roll(qw, D // 2)[None, :]).reshape(N_ST, 128, D))
    cwk = np.ascontiguousarray((cos * kw[None, :]).reshape(N_ST, 128, D))
    swk = np.ascontiguousarray((sin * np.roll(kw, D // 2)[None, :]).reshape(N_ST, 128, D))

    in_maps = []
    for c in range(NC):
        wq_c = Wq[c * DQ : (c + 1) * DQ, :]  # [DQ, HID]
        wq_t = np.ascontiguousarray(wq_c.T.reshape(HC, 128, DQ).astype(bf))
        wk_c = Wk[c * D : (c + 1) * D, :]
        wv_c = Wv[c * D : (c + 1) * D, :]
        wkv_t = np.ascontiguousarray(
            np.concatenate([wk_c.T, wv_c.T], axis=1).reshape(HC, 128, 2 * D).astype(bf)
        )
        wo_c = Wo[:, c * DQ : (c + 1) * DQ]  # [HID, DQ]
        wo_t = np.ascontiguousarray(
            wo_c.T.reshape(G, 128, HID).transpose(1, 0, 2).astype(bf)
        )
        in_maps.append(
            {
                "xt": xt,
                "wq": wq_t,
                "wkv": wkv_t,
                "wo": wo_t,
                "cwq": cwq,
                "swq": swq,
                "cwk": cwk,
                "swk": swk,
            }
        )
    return in_maps


def assemble(outs):
    # outs[c] is [S//NC, HID] bf16. RS chunk qc covers global rows
    # [512*qc, +512); core c receives rows [64*c, 64*c+64) of it,
    # stored at core-local rows [64*qc, +64).
    y = np.empty((S, HID), dtype=np.float32)
    rows = QC // NC  # 64
    for qc in range(N_QC):
        for c in range(NC):
            g0 = QC * qc + rows * c
            l0 = rows * qc
            y[g0 : g0 + rows, :] = outs[c][l0 : l0 + rows, :].astype(np.float32)
    return y.reshape(B, S, HID)


def kernel(**inputs) -> np.ndarray:
    nc = _get_nc()
    in_maps = make_in_maps(inputs)
    res = bass_utils.run_bass_kernel_spmd(nc, in_maps, core_ids=list(range(NC)))
    return assemble([r["out"] for r in res.results])


# revision 33
# speedup vs baseline: 1.0429x; 1.0429x over previous
# GQA attention layer (B=1, S=2048, HID=2560, H=32, HKV=8, D=128) on 8 TRN2
# NeuronCores. Tensor-parallel over kv-head groups: core c owns kv head c and
# its 4 query heads (Wq/Wk/Wv row shards, Wo column shard). The o_proj
# partials are combined with an on-device ReduceScatter over the sequence
# axis (4 chunks, overlapped with compute); the host reassembles the
# sequence-sharded outputs.
#
# Per-core dataflow (all matmuls bf16 -> fp32 PSUM):
#   1. QKV projection from X^T tiles (s-major output layout), per-head
#      RMSNorm + RoPE on DVE, PE-transpose of Q/K into [d, s] layout.
#      rsqrt(var) is computed as exp(-0.5*ln(var)) on ACT so every ACT
#      function used by the kernel ({Ln, Exp, Copy}) lives in one act table
#      (no table reloads).
#   2. Scores are computed transposed (S^T[k, q] = K Q^T) so that the
#      P^T @ V matmul needs no transpose of the 16.8M-element prob matrix.
#      Scores land in 2-bank PSUM tiles (two 512-col matmuls) so one ACT
#      exp instruction covers 1024 columns. Softmax denominators: a DVE
#      binary tree compresses exp tiles 16 -> 1 along kt, then a single
#      all-ones [128,128] stationary matmul broadcasts the column sums to
#      every partition (vs 16 ones-matmuls = 67us of PE in the baseline).
#      reciprocal_approx_fast (~18 bits) inverts the sums.
#   3. o_proj per 512-row chunk (evictions all on DVE), ReduceScatter per
#      chunk in bf16 overlapped with the next chunk's compute. The
#      rs_out -> out DMAs are emitted at the END of the program so a
#      collective-waiting DMA never parks at the head of the sync queue and
#      head-of-line-blocks the next chunk's eviction DMAs (a 44us PE stall
#      in the baseline).
import sys

if "/opt/trn_rl_repo" not in sys.path:
    sys.path.insert(0, "/opt/trn_rl_repo")

import numpy as np
import ml_dtypes

import concourse.bacc as bacc
import concourse.mybir as mybir
import concourse.tile as tile
from concourse import bass_utils, masks

BF16 = mybir.dt.bfloat16
F32 = mybir.dt.float32

B, S, HID = 1, 2048, 2560
H, HKV, D = 32, 8, 128
G = H // HKV  # q heads per kv head (= per core)
NC = 8  # cores
DQ = G * D  # per-core q width (512)
EPS = 1e-6
SCALE = 1.0 / float(np.sqrt(D))

ST = 128          # s positions per compute tile
N_ST = S // ST    # 16
HC = HID // 128   # 20 contraction chunks
XL = 256          # s positions per X^T DMA load tile
N_XL = S // XL    # 8
QC = 512          # q positions per attention unit
N_QC = S // QC    # 4 (also the ReduceScatter chunk count)
N_KT = S // 128   # 16 k tiles per attention unit
N_KP = N_KT // 2  # 8 kt pairs (exp granularity)
NO = HID // 512   # 5 o_proj free-dim chunks

_NC_CACHE = None


def _build(reps: int = 1, single: bool = False):
    nc = bacc.Bacc(
        "TRN2", target_bir_lowering=False, debug=False,
        num_devices=(1 if single else NC),
    )

    # partition-major X^T tiles: each partition's line is HC*XL*2 = 10 KB
    # contiguous, so the load DMAs run at full descriptor efficiency
    xt_d = nc.dram_tensor("xt", [N_XL, 128, HC, XL], BF16, kind="ExternalInput").ap()
    wq_d = nc.dram_tensor("wq", [HC, 128, DQ], BF16, kind="ExternalInput").ap()
    wkv_d = nc.dram_tensor("wkv", [HC, 128, 2 * D], BF16, kind="ExternalInput").ap()
    wo_d = nc.dram_tensor("wo", [128, G, HID], BF16, kind="ExternalInput").ap()
    cwq_d = nc.dram_tensor("cwq", [N_ST, 128, D], F32, kind="ExternalInput").ap()
    swq_d = nc.dram_tensor("swq", [N_ST, 128, D], F32, kind="ExternalInput").ap()
    cwk_d = nc.dram_tensor("cwk", [N_ST, 128, D], F32, kind="ExternalInput").ap()
    swk_d = nc.dram_tensor("swk", [N_ST, 128, D], F32, kind="ExternalInput").ap()
    out_d = nc.dram_tensor("out", [S // NC, HID], BF16, kind="ExternalOutput").ap()

    with tile.TileContext(nc) as tc:
        with (
            tc.tile_pool(name="const", bufs=1) as cpool,
            tc.tile_pool(name="xt", bufs=2) as xt_pool,
            tc.tile_pool(name="cs", bufs=8) as cs_pool,
            tc.tile_pool(name="qw", bufs=5) as qw_pool,
            tc.tile_pool(name="kw", bufs=6) as kw_pool,
            tc.tile_pool(name="ro", bufs=2) as ro_pool,
            tc.tile_pool(name="sm", bufs=4) as sm_pool,
            tc.tile_pool(name="ep", bufs=2) as ep_pool,
            tc.tile_pool(name="tr", bufs=2) as tr_pool,
            tc.tile_pool(name="ot", bufs=8) as ot_pool,
            tc.tile_pool(name="ob", bufs=4) as ob_pool,
            tc.tile_pool(name="psA", bufs=2, space="PSUM") as psA,
            tc.tile_pool(name="psB", bufs=2, space="PSUM") as psB,
            tc.tile_pool(name="psC", bufs=2, space="PSUM") as psC,
            tc.tile_pool(name="dram", bufs=1, space="DRAM") as dram,
        ):
            for _rep in range(reps):
                # Pin the ACT table to natural_log_exp_and_others (id 6 in
                # act_info.json): it contains every ACT function this kernel
                # uses ({Ln, Exp, Copy}), so the compiler's table-load pass
                # finds the table already resident on every path and inserts
                # no per-iteration reloads (the baseline paid ~27 1.3us
                # reloads thrashing between the sqrt and exp tables).
                nc.scalar.add_instruction(
                    mybir.InstLoadActFuncSet(
                        name=nc.get_next_instruction_name(),
                        ins=[],
                        outs=[],
                        act_func_set_id=6,
                    )
                )
                # ---- resident constants / weights ----
                ident = cpool.tile([128, 128], BF16, tag="ident")
                masks.make_identity(nc, ident[:])
                # all-ones stationary: one matmul on the kt-compressed exp
                # sums yields the softmax denominator replicated across all
                # 128 partitions (free bcast)
                ones_k = cpool.tile([128, 128], BF16, tag="ones_k")
                nc.vector.memset(ones_k[:], 1.0)

                # first X^T tile before the weight block so the PE can start
                # as soon as xt[0] + wq[0] land; split in half so the first
                # ST only waits on 0.65 MB
                xt_t = xt_pool.tile([128, HC, XL], BF16, tag="xt")
                nc.sync.dma_start(xt_t[:], xt_d[0])

                # per-chunk weight tiles so the first matmul only waits on
                # chunk 0, not the whole 6.5 MB weight load
                wq_t = []
                wkv_t = []
                xt_next = None
                for ch in range(HC):
                    w1 = cpool.tile([128, DQ], BF16, tag=f"wq{ch}")
                    nc.sync.dma_start(w1[:], wq_d[ch])
                    wq_t.append(w1)
                    w2 = cpool.tile([128, 2 * D], BF16, tag=f"wkv{ch}")
                    nc.sync.dma_start(w2[:], wkv_d[ch])
                    wkv_t.append(w2)
                    if ch == 5:
                        xt_next = xt_pool.tile([128, HC, XL], BF16, tag="xt")
                        nc.sync.dma_start(xt_next[:], xt_d[1])

                qt_sb = cpool.tile([128, G, S], BF16, tag="qt")   # Q^T  [d, h, s]
                kt_sb = cpool.tile([128, S], BF16, tag="kt")      # K^T  [d, s]
                v_sb = cpool.tile([128, N_KT, D], BF16, tag="v")  # V    [s%128, kt, d]

                # ================= phase 1: QKV + norm + rope + transpose ======
                # first attention unit's ep tile; its scores/exp are emitted
                # inside the phase-1 loop as soon as each kt PAIR's K^T lands
                ep00 = ep_pool.tile([128, N_KT, QC], BF16, tag="ep")
                t8_00 = tr_pool.tile([128, N_KP, QC], BF16, tag="t8")
                for st in range(N_ST):
                    if st % (XL // ST) == 0 and st > 0:
                        if st // (XL // ST) == 1:
                            xt_t = xt_next
                        else:
                            xt_t = xt_pool.tile([128, HC, XL], BF16, tag="xt")
                            nc.sync.dma_start(xt_t[:], xt_d[st // (XL // ST)])
                    soff = (st % (XL // ST)) * ST

                    cwq_t = cs_pool.tile([128, D], F32, tag="cs")
                    nc.sync.dma_start(cwq_t[:], cwq_d[st])
                    swq_t = cs_pool.tile([128, D], F32, tag="cs")
                    nc.sync.dma_start(swq_t[:], swq_d[st])
                    cwk_t = cs_pool.tile([128, D], F32, tag="cs")
                    nc.sync.dma_start(cwk_t[:], cwk_d[st])
                    swk_t = cs_pool.tile([128, D], F32, tag="cs")
                    nc.sync.dma_start(swk_t[:], swk_d[st])

                    q_ps = psA.tile([128, DQ], F32, tag="s2")
                    kv_ps = psB.tile([128, 2 * D], F32, tag="b")
                    for ch in range(HC):
                        lhs = xt_t[:, ch, soff : soff + ST]
                        nc.tensor.matmul(
                            q_ps[:], lhs, wq_t[ch][:],
                            start=(ch == 0), stop=(ch == HC - 1),
                        )
                        nc.tensor.matmul(
                            kv_ps[:], lhs, wkv_t[ch][:],
                            start=(ch == 0), stop=(ch == HC - 1),
                        )

                    # evictions (scalar engine)
                    q_sb = qw_pool.tile([128, DQ], F32, tag="qw")
                    nc.scalar.copy(q_sb[:], q_ps[:])
                    k_sb = kw_pool.tile([128, D], F32, tag="kw")
                    nc.scalar.copy(k_sb[:], kv_ps[:, 0:D])
                    nc.scalar.copy(v_sb[:, st, :], kv_ps[:, D : 2 * D])

                    # ---- RMSNorm (per head) ----
                    sq = qw_pool.tile([128, DQ], F32, tag="qw")
                    nc.vector.tensor_mul(sq[:], q_sb[:], q_sb[:])
                    ssq = sm_pool.tile([128, G + 1], F32, tag="sm")
                    nc.vector.tensor_reduce(
                        ssq[:, 0:G], sq[:].rearrange("p (h d) -> p h d", d=D),
                        axis=mybir.AxisListType.X, op=mybir.AluOpType.add,
                    )
                    ksq = kw_pool.tile([128, D], F32, tag="kw")
                    nc.vector.tensor_mul(ksq[:], k_sb[:], k_sb[:])
                    nc.vector.tensor_reduce(
                        ssq[:, G : G + 1], ksq[:].unsqueeze(1),
                        axis=mybir.AxisListType.X, op=mybir.AluOpType.add,
                    )
                    var = sm_pool.tile([128, G + 1], F32, tag="sm")
                    nc.vector.tensor_scalar(
                        var[:], ssq[:], 1.0 / D, EPS,
                        op0=mybir.AluOpType.mult, op1=mybir.AluOpType.add,
                    )
                    # rsqrt(var) = exp(-0.5 * ln(var)): keeps ACT on the
                    # {Ln, Exp, Copy} table set (no act-table reloads)
                    lnv = sm_pool.tile([128, G + 1], F32, tag="sm")
                    nc.scalar.activation(
                        lnv[:], var[:], mybir.ActivationFunctionType.Ln
                    )
                    rq = sm_pool.tile([128, G + 1], F32, tag="sm")
                    nc.scalar.activation(
                        rq[:], lnv[:], mybir.ActivationFunctionType.Exp,
                        scale=-0.5,
                    )
                    rk = rq

                    # ---- normalize + rope (DVE) ----
                    qn = qw_pool.tile([128, DQ], F32, tag="qw")
                    qn3 = qn[:].rearrange("p (h d) -> p h d", d=D)
                    nc.vector.tensor_tensor(
                        qn3, q_sb[:].rearrange("p (h d) -> p h d", d=D),
                        rq[:, 0:G].unsqueeze(2).to_broadcast([128, G, D]),
                        op=mybir.AluOpType.mult,
                    )
                    t1 = qw_pool.tile([128, DQ], F32, tag="qw")
                    t13 = t1[:].rearrange("p (h d) -> p h d", d=D)
                    cwq3 = cwq_t[:].unsqueeze(1).to_broadcast([128, G, D])
                    swq3 = swq_t[:].unsqueeze(1).to_broadcast([128, G, D])
                    nc.vector.tensor_tensor(t13, qn3, cwq3, op=mybir.AluOpType.mult)
                    u = qw_pool.tile([128, DQ], F32, tag="qw")
                    u3 = u[:].rearrange("p (h d) -> p h d", d=D)
                    hd = D // 2
                    nc.vector.tensor_tensor(
                        u3[:, :, 0:hd], qn3[:, :, hd:D], swq3[:, :, 0:hd],
                        op=mybir.AluOpType.mult,
                    )
                    nc.vector.tensor_tensor(
                        u3[:, :, hd:D], qn3[:, :, 0:hd], swq3[:, :, hd:D],
                        op=mybir.AluOpType.mult,
                    )
                    qro = ro_pool.tile([128, DQ], BF16, tag="qro")
                    qro3 = qro[:].rearrange("p (h d) -> p h d", d=D)
                    nc.vector.tensor_sub(qro3[:, :, 0:hd], t13[:, :, 0:hd], u3[:, :, 0:hd])
                    nc.vector.tensor_add(qro3[:, :, hd:D], t13[:, :, hd:D], u3[:, :, hd:D])

                    kn = kw_pool.tile([128, D], F32, tag="kw")
                    nc.vector.tensor_tensor(
                        kn[:], k_sb[:],
                        rk[:, G : G + 1].to_broadcast([128, D]),
                        op=mybir.AluOpType.mult,
                    )
                    kt1 = kw_pool.tile([128, D], F32, tag="kw")
                    nc.vector.tensor_tensor(kt1[:], kn[:], cwk_t[:], op=mybir.AluOpType.mult)
                    ku = kw_pool.tile([128, D], F32, tag="kw")
                    nc.vector.tensor_tensor(
                        ku[:, 0:hd], kn[:, hd:D], swk_t[:, 0:hd], op=mybir.AluOpType.mult
                    )
                    nc.vector.tensor_tensor(
                        ku[:, hd:D], kn[:, 0:hd], swk_t[:, hd:D], op=mybir.AluOpType.mult
                    )
                    kro = ro_pool.tile([128, D], BF16, tag="kro")
                    nc.vector.tensor_sub(kro[:, 0:hd], kt1[:, 0:hd], ku[:, 0:hd])
                    nc.vector.tensor_add(kro[:, hd:D], kt1[:, hd:D], ku[:, hd:D])

                    # ---- transpose Q heads + K into [d, s] ----
                    for h in range(G):
                        tp = psC.tile([128, 128], BF16, tag="c")
                        nc.tensor.transpose(tp[:], qro[:, h * D : (h + 1) * D], ident[:])
                        nc.scalar.copy(qt_sb[:, h, st * ST : (st + 1) * ST], tp[:])
                    tp = psC.tile([128, 128], BF16, tag="c")
                    nc.tensor.transpose(tp[:], kro[:], ident[:])
                    nc.scalar.copy(kt_sb[:, st * ST : (st + 1) * ST], tp[:])

                    # interleave unit (0,0)'s scores+exp per completed kt PAIR
                    if st >= 5 and st % 2 == 1:
                        kp = (st - 5) // 2
                        s_ps = psA.tile([128, 2 * QC], F32, tag="s2")
                        for j in range(2):
                            kt = 2 * kp + j
                            nc.tensor.matmul(
                                s_ps[:, j * QC : (j + 1) * QC],
                                kt_sb[:, kt * 128 : (kt + 1) * 128],
                                qt_sb[:, 0, 0:QC],
                                start=True, stop=True,
                            )
                        nc.scalar.activation(
                            ep00[:].rearrange("p k q -> p (k q)")[
                                :, 2 * kp * QC : (2 * kp + 2) * QC
                            ],
                            s_ps[:],
                            mybir.ActivationFunctionType.Exp, scale=SCALE,
                        )
                        with nc.allow_low_precision("bf16 softmax partials"):
                            nc.vector.tensor_add(
                                t8_00[:, kp, :],
                                ep00[:, 2 * kp, :], ep00[:, 2 * kp + 1, :],
                            )
                            if kp == 3:
                                nc.vector.tensor_add(
                                    t8_00[:, 0:2, :],
                                    t8_00[:, 0:2, :], t8_00[:, 2:4, :],
                                )
                            if kp == 5:
                                nc.vector.tensor_add(
                                    t8_00[:, 0, :], t8_00[:, 0, :], t8_00[:, 1, :]
                                )
                                nc.vector.tensor_add(
                                    t8_00[:, 4, :], t8_00[:, 4, :], t8_00[:, 5, :]
                                )

                # ================= phase 2: attention + o_proj + RS ============
                # wo is first needed ~10us into phase 2; load it behind the
                # phase-1 traffic instead of ahead of it
                wo_sb = cpool.tile([128, G, HID], BF16, tag="wo")
                nc.sync.dma_start(wo_sb[:], wo_d)
                for qc in range(N_QC):
                    ot_tiles = []
                    for h in range(G):
                        if qc == 0 and h == 0:
                            ep = ep00
                            t8 = t8_00
                            kp_start = N_KP - 2
                        else:
                            ep = ep_pool.tile([128, N_KT, QC], BF16, tag="ep")
                            t8 = tr_pool.tile([128, N_KP, QC], BF16, tag="t8")
                            kp_start = 0
                        epf = ep[:].rearrange("p k q -> p (k q)")
                        for kp in range(kp_start, N_KP):
                            s_ps = psA.tile([128, 2 * QC], F32, tag="s2")
                            for j in range(2):
                                kt = 2 * kp + j
                                nc.tensor.matmul(
                                    s_ps[:, j * QC : (j + 1) * QC],
                                    kt_sb[:, kt * 128 : (kt + 1) * 128],
                                    qt_sb[:, h, qc * QC : (qc + 1) * QC],
                                    start=True, stop=True,
                                )
                            nc.scalar.activation(
                                epf[:, 2 * kp * QC : (2 * kp + 2) * QC],
                                s_ps[:],
                                mybir.ActivationFunctionType.Exp, scale=SCALE,
                            )
                            # incremental pair-sum right after each exp, with
                            # progressive folds at kp=3/5: after the LAST exp
                            # only 3 small adds remain before the sums matmul,
                            # so the denominator chain no longer head-blocks
                            # the PE queue ahead of each chunk's o_proj
                            with nc.allow_low_precision("bf16 softmax partials"):
                                nc.vector.tensor_add(
                                    t8[:, kp, :],
                                    ep[:, 2 * kp, :], ep[:, 2 * kp + 1, :],
                                )
                                if kp == 3:
                                    # pairs 0-3 -> 2 slices
                                    nc.vector.tensor_add(
                                        t8[:, 0:2, :], t8[:, 0:2, :], t8[:, 2:4, :]
                                    )
                                if kp == 5:
                                    # pairs 0-3 -> 1 slice; pairs 4,5 -> 1
                                    nc.vector.tensor_add(
                                        t8[:, 0, :], t8[:, 0, :], t8[:, 1, :]
                                    )
                                    nc.vector.tensor_add(
                                        t8[:, 4, :], t8[:, 4, :], t8[:, 5, :]
                                    )
                        pv_ps = psB.tile([128, QC], F32, tag="b")
                        for kt in range(N_KT):
                            nc.tensor.matmul(
                                pv_ps[:], v_sb[:, kt, :], ep[:, kt, :],
                                start=(kt == 0), stop=(kt == N_KT - 1),
                            )
                        # finish the denominator: slices 0 (pairs 0-3),
                        # 4 (pairs 4,5), 6, 7 remain after the in-loop folds
                        with nc.allow_low_precision(
                            "bf16 partial sums of positive exp values; "
                            "relative error ~0.3% on the softmax denominator"
                        ):
                            nc.vector.tensor_add(
                                t8[:, 6, :], t8[:, 6, :], t8[:, 7, :]
                            )
                            nc.vector.tensor_add(
                                t8[:, 4, :], t8[:, 4, :], t8[:, 6, :]
                            )
                            nc.vector.tensor_add(
                                t8[:, 0, :], t8[:, 0, :], t8[:, 4, :]
                            )
                        sums_ps = psC.tile([128, QC], F32, tag="c")
                        nc.tensor.matmul(
                            sums_ps[:], ones_k[:], t8[:, 0, :],
                            start=True, stop=True,
                        )
                        # sums_ps rows are all identical (ones stationary) —
                        # approx reciprocal (~18 bits) is plenty for softmax
                        rb = sm_pool.tile([128, QC], F32, tag="rb", bufs=2)
                        nc.vector.reciprocal_approx_fast(rb[:], sums_ps[:])
                        ot = ot_pool.tile([128, QC], BF16, tag="ot")
                        nc.vector.tensor_tensor(
                            ot[:], pv_ps[:], rb[:], op=mybir.AluOpType.mult
                        )
                        ot_tiles.append(ot)

                    # o_proj for this 512-row chunk; one full-chunk
                    # ReduceScatter (2.6 MB/rank -> RDH regime, better bus rate)
                    RROWS = QC // NC  # 64 output rows per core per RS
                    rs_in = dram.tile([QC, HID], BF16, tag=f"rsin{qc}")
                    rs_out = dram.tile([RROWS, HID], BF16, tag=f"rsout{qc}")
                    for si in range(QC // ST):
                        sst = si
                        ob = ob_pool.tile([128, HID], BF16, tag="ob")
                        for no in range(NO):
                            y_ps = psB.tile([128, 512], F32, tag="b")
                            for h in range(G):
                                nc.tensor.matmul(
                                    y_ps[:],
                                    ot_tiles[h][:, sst * ST : (sst + 1) * ST],
                                    wo_sb[:, h, no * 512 : (no + 1) * 512],
                                    start=(h == 0), stop=(h == G - 1),
                                )
                            nc.vector.tensor_copy(
                                ob[:, no * 512 : (no + 1) * 512], y_ps[:]
                            )
                            nc.sync.dma_start(
                                rs_in[si * ST : (si + 1) * ST,
                                      no * 512 : (no + 1) * 512],
                                ob[:, no * 512 : (no + 1) * 512],
                            )

                    orow = qc * (QC // NC)
                    if single:
                        nc.sync.dma_start(
                            out_d[orow : orow + RROWS, :], rs_in[0:RROWS, :]
                        )
                    else:
                        nc.gpsimd.collective_compute(
                            "ReduceScatter",
                            mybir.AluOpType.add,
                            replica_groups=[list(range(NC))],
                            ins=[rs_in.opt()],
                            outs=[rs_out.opt()],
                        )
                        # rs_out -> out DMA on the GPSIMD queue: it waits for
                        # the collective, and the only things behind it there
                        # are later collectives (which the in-order CC stream
                        # serializes anyway). On the sync queue this DMA
                        # head-of-line-blocked the next chunk's o_proj
                        # eviction DMAs -> 44us PE stall in the baseline.
                        nc.gpsimd.dma_start(
                            out_d[orow : orow + RROWS, :], rs_out[:]
                        )

    nc.compile()
    return nc


def _get_nc():
    global _NC_CACHE
    if _NC_CACHE is None:
        _NC_CACHE = _build()
    return _NC_CACHE


def make_in_maps(inputs):
    X = np.asarray(inputs["hidden_states"], dtype=np.float32).reshape(S, HID)
    freqs = np.asarray(inputs["freqs_cis"], dtype=np.float32)
    Wq = np.asarray(inputs["Wq"], dtype=np.float32)
    Wk = np.asarray(inputs["Wk"], dtype=np.float32)
    Wv = np.asarray(inputs["Wv"], dtype=np.float32)
    Wo = np.asarray(inputs["Wo"], dtype=np.float32)
    qw = np.asarray(inputs["q_norm_w"], dtype=np.float32)
    kw = np.asarray(inputs["k_norm_w"], dtype=np.float32)

    bf = ml_dtypes.bfloat16
    # X^T load tiles, partition-major: (L, p, ch, s) = X[L*XL+s, ch*128+p]
    xt = np.ascontiguousarray(
        X.reshape(N_XL, XL, HC, 128).transpose(0, 3, 2, 1).astype(bf)
    )
    cos, sin = freqs[0], freqs[1]  # [S, D]
    cwq = np.ascontiguousarray((cos * qw[None, :]).reshape(N_ST, 128, D))
    swq = np.ascontiguousarray((sin * np.roll(qw, D // 2)[None, :]).reshape(N_ST, 128, D))
    cwk = np.ascontiguousarray((cos * kw[None, :]).reshape(N_ST, 128, D))
    swk = np.ascontiguousarray((sin * np.roll(kw, D // 2)[None, :]).reshape(N_ST, 128, D))

    in_maps = []
    for c in range(NC):
        wq_c = Wq[c * DQ : (c + 1) * DQ, :]  # [DQ, HID]
        wq_t = np.ascontiguousarray(wq_c.T.reshape(HC, 128, DQ).astype(bf))
        wk_c = Wk[c * D : (c + 1) * D, :]
        wv_c = Wv[c * D : (c + 1) * D, :]
        wkv_t = np.ascontiguousarray(
            np.concatenate([wk_c.T, wv_c.T], axis=1).reshape(HC, 128, 2 * D).astype(bf)
        )
        wo_c = Wo[:, c * DQ : (c + 1) * DQ]  # [HID, DQ]
        wo_t = np.ascontiguousarray(
            wo_c.T.reshape(G, 128, HID).transpose(1, 0, 2).astype(bf)
        )
        in_maps.append(
            {
                "xt": xt,
                "wq": wq_t,
                "wkv": wkv_t,
                "wo": wo_t,
                "cwq": cwq,
                "swq": swq,
                "cwk": cwk,
                "swk": swk,
            }
        )
    return in_maps


def assemble(outs):
    # outs[c] is [S//NC, HID] bf16. RS chunk qc covers global rows
    # [512*qc, +512); core c receives rows [64*c, 64*c+64) of it,
    # stored at core-local rows [64*qc, +64).
    y = np.empty((S, HID), dtype=np.float32)
    rows = QC // NC  # 64
    for qc in range(N_QC):
        for c in range(NC):
            g0 = QC * qc + rows * c
            l0 = rows * qc
            y[g0 : g0 + rows, :] = outs[c][l0 : l0 + rows, :].astype(np.float32)
    return y.reshape(B, S, HID)


def kernel(**inputs) -> np.ndarray:
    nc = _get_nc()
    in_maps = make_in_maps(inputs)
    res = bass_utils.run_bass_kernel_spmd(nc, in_maps, core_ids=list(range(NC)))
    return assemble([r["out"] for r in res.results])
